# revision 38
# baseline (speedup 1.0000x reference)
"""Stride-2 bilinear upsampling (block-diagonal conv_transpose2d) on 8 NeuronCores.

v5 design: bf16 end-to-end, column-planar device output, host-side scalar
preprocessing, and a fused H-pass.

Device I/O (per core, 128 images one-per-partition):
  in : x  = a^2 * X   padded to the S layout [128, 128, 132] bf16
       t3 = 3 a^2 * X padded likewise        (pure scalar preprocessing --
       the 4-tap filter is a*[1,3,3,1]; shipping 3S beside S removes the
       on-device T=3S pass entirely)
  out: out_e / out_o [128, 258, 130] bf16 -- even/odd column planes (the
       host interleaves columns during the unshard and upcasts to f32).

SBUF layout: per parity plane a combined tile UV [130, 260] holds the
W-upsampled image U in cols 0:130 and V = 3U in cols 130:260, row-adjacent
so the H-pass collapses to ONE packed-2x tensor_tensor per strip-plane:

  Z[2i]   = V[m0+i] + U[m0+i+1]      in0 = pairs (V[r], U[r])  r = m0..
  Z[2i+1] = U[m0+i] + V[m0+i+1]      in1 = flat rows (U[r],V[r]) r = m0+1..

in0 is a hand-built access pattern [[260,n],[-130,2],[1,130]] starting at
(row m0, col 130); in1 is just the flat row slice.  All tensor_tensor ops
are packed-innermost (the bf16 2x uop, ~0.52 ns/elem); V runs on ACT
(stride-blind).  GpSimd does no compute (its software tt trashes SBUF
bandwidth for everyone) -- it serves as the second DMA ring:
sync = S-in + even-plane out, gpsimd = T-in + odd-plane out.
"""

import numpy as np

N, C, H, W = 4, 256, 128, 128
OH, OW = 258, 258
PW = OW // 2 + 1           # plane width padded to 130 (even, 4B-aligned rows)
NCORES = 8
CPC = C // NCORES          # 32 channels per core
NIMG = N * CPC             # 128 images per core (one per SBUF partition)
SW = W + 4                 # S width: [pad][ghostL][128 data][ghostR][pad]

_CACHE = {}


def _legalize_waits(nc, mybir):
    """Split multi-wait sync_info into standalone single-wait EventSemaphore
    instructions (this build encodes at most one sync-wait per instruction)."""
    n = 0
    for func in nc.m.functions:
        for block in func.blocks:
            out = []
            for inst in block.instructions:
                si = inst.sync_info
                if si is not None and si.on_wait is not None and len(si.on_wait) > 1:
                    waits = list(si.on_wait)
                    for k, w in enumerate(waits[:-1]):
                        out.append(mybir.InstEventSemaphore(
                            name=f"{inst.name}-hw{k}",
                            opcode="EventSemaphore",
                            engine=inst.engine,
                            ins=[], outs=[],
                            sync_info=mybir.SyncInfo(on_wait=[w], on_update=[]),
                        ))
                        n += 1
                    inst.sync_info = mybir.SyncInfo(
                        on_wait=[waits[-1]], on_update=list(si.on_update))
                out.append(inst)
            block.instructions = out
    return n


def _build_bass(strips=None, bufs_c=2, bufs_z=3, h_fuse=1, v_dve_edge=1,
                v_act=1):
    """Per-core: x/t3 [128,128,132] bf16 -> out_e/out_o [128,258,130] bf16."""
    import concourse.bass as bass
    import concourse.mybir as mybir
    from concourse.tile import TileContext

    bf16 = mybir.dt.bfloat16
    Copy = mybir.ActivationFunctionType.Copy
    add = mybir.AluOpType.add
    if strips is None:
        strips = [8] + [24] * 4 + [16, 8]
    assert sum(strips) == H
    nstrips = len(strips)
    hs_max = max(strips)
    m0s = list(np.cumsum([0] + strips[:-1]))

    nc = bass.Bass()
    x = nc.dram_tensor("x", [NIMG, H, SW], bf16, kind="ExternalInput")
    out_e = nc.dram_tensor("out_e", [NIMG, OH, PW], bf16,
                           kind="ExternalOutput")
    out_o = nc.dram_tensor("out_o", [NIMG, OH, PW], bf16,
                           kind="ExternalOutput")

    with TileContext(nc) as tc:
        with tc.tile_pool(name="p", bufs=2) as pool:
            # rolling U|V strip tiles (one per parity plane per strip):
            # local row r = global row m0 + r.  Strips recompute the 1-row
            # overlap (chunks overlap one x row) so there are no
            # cross-strip UV dependencies at all.
            uvs = {}

            def emit_h(s):
                hs = strips[s]
                m0 = int(m0s[s])
                n_m = hs + (1 if s == nstrips - 1 else 0)
                for par, UV, dram in ((0, uvs[s][0], out_e),
                                      (1, uvs[s][1], out_o)):
                    # H-pass: Z rows 2m0 .. 2(m0+n_m)-1
                    zt_full = pool.tile([NIMG, 2 * hs_max + 2, PW], bf16,
                                        tag="zt", bufs=bufs_z)
                    zt = zt_full[:, 0:2 * n_m, :]
                    if h_fuse:
                        # in0: pairs (V[r], U[r]) for local r = 0..n_m-1
                        in0 = UV[:, 0:n_m, :].copy()
                        in0.ap[2] = [-PW, 2]
                        in0.ap.append([1, PW])
                        in0.offset = in0.offset + PW
                        nc.vector.tensor_tensor(
                            out=zt, in0=in0,
                            in1=UV[:, 1:n_m + 1, :], op=add)
                    else:
                        nc.vector.tensor_tensor(
                            out=zt[:, 0:2 * n_m:2, :],
                            in0=UV[:, 0:n_m, PW:2 * PW],
                            in1=UV[:, 1:n_m + 1, 0:PW], op=add)
                        nc.vector.tensor_tensor(
                            out=zt[:, 1:2 * n_m:2, :],
                            in0=UV[:, 0:n_m, 0:PW],
                            in1=UV[:, 1:n_m + 1, PW:2 * PW], op=add)
                    oeng = nc.gpsimd if par == 1 else nc.sync
                    oeng.dma_start(out=dram[:, 2 * m0:2 * (m0 + n_m), :],
                                   in_=zt)

            # software-pipelined emission with staged prefetch: at loop
            # iteration s we issue the chunk DMA for strip s+2, T=3S for
            # strip s+1, W+V for strip s, and the H-pass for strip s-1 --
            # so neither the input DMA nor the ACT T/V latency ever
            # bubbles the DVE queue.
            sts, tts = {}, {}

            def emit_dma(s):
                hs = strips[s]
                m0 = int(m0s[s])
                r0, nr = (0, hs) if s == 0 else (m0 - 1, hs + 1)
                st = pool.tile([NIMG, hs_max + 1, SW], bf16, tag="st",
                               bufs=bufs_c)
                nc.sync.dma_start(out=st[:, 0:nr, :], in_=x[:, r0:r0 + nr, :])
                sts[s] = st

            def emit_t(s):
                hs = strips[s]
                nr = hs if s == 0 else hs + 1
                edge = s < v_dve_edge or s >= nstrips - v_dve_edge
                tt_ = pool.tile([NIMG, hs_max + 1, SW], bf16, tag="tt",
                                bufs=bufs_c)
                if v_act and not edge:
                    nc.scalar.activation(tt_[:, 0:nr, :], sts[s][:, 0:nr, :],
                                         Copy, scale=3.0)
                else:
                    nc.vector.tensor_scalar_mul(tt_[:, 0:nr, :],
                                                sts[s][:, 0:nr, :], 3.0)
                tts[s] = tt_

            emit_dma(0)
            emit_dma(1)
            emit_t(0)
            for s in range(nstrips):
                hs = strips[s]
                m0 = int(m0s[s])
                n_m = hs + (1 if s == nstrips - 1 else 0)
                edge = s < v_dve_edge or s >= nstrips - v_dve_edge
                if s + 2 < nstrips:
                    emit_dma(s + 2)
                if s + 1 < nstrips:
                    emit_t(s + 1)

                st, tt_ = sts.pop(s), tts.pop(s)
                UVe = pool.tile([NIMG, hs_max + 2, 2 * PW], bf16, tag="uve",
                                bufs=3)
                UVo = pool.tile([NIMG, hs_max + 2, 2 * PW], bf16, tag="uvo",
                                bufs=3)
                uvs[s] = (UVe, UVo)
                last = s == nstrips - 1
                # local U rows to compute: 1..hs for strip 0 (row 0 is the
                # ghost), else 0..hs (recompute the overlap row)
                lo = 1 if s == 0 else 0
                co = 0 if s == 0 else 1   # chunk rows run one earlier
                nw = hs + 1 - lo
                if s == 0:
                    for UV in (UVe, UVo):
                        nc.gpsimd.memset(UV[:, 0:1, :], 0.0)
                if last:
                    for UV in (UVe, UVo):
                        nc.vector.memset(UV[:, n_m:n_m + 1, :], 0.0)
                # W-pass (packed tt 2x): local U[r] <- S/T chunk row r-lo
                #   Ue = T[j+1] + S[j+2];  Uo = S[j+1] + T[j+2]
                nc.vector.tensor_tensor(
                    out=UVe[:, lo:hs + 1, 0:PW],
                    in0=tt_[:, 0:nw, 1:W + 3],
                    in1=st[:, 0:nw, 2:W + 4], op=add)
                nc.vector.tensor_tensor(
                    out=UVo[:, lo:hs + 1, 0:PW],
                    in0=st[:, 0:nw, 1:W + 3],
                    in1=tt_[:, 0:nw, 2:W + 4], op=add)

                for par, UV in ((0, UVe), (1, UVo)):
                    # V = 3U local rows lo..n_m (incl. the overlap row; the
                    # last strip's row n_m reads the memset ghost)
                    nv = n_m + 1 - lo if last else hs + 1 - lo
                    vdst = UV[:, lo:lo + nv, PW:2 * PW]
                    vsrc = UV[:, lo:lo + nv, 0:PW]
                    if v_act and not edge:
                        nc.scalar.activation(vdst, vsrc, Copy, scale=3.0)
                    else:
                        nc.vector.tensor_scalar_mul(vdst, vsrc, 3.0)

                if s > 0:
                    emit_h(s - 1)
            emit_h(nstrips - 1)

    _legalize_waits(nc, mybir)
    return nc


def _taps_from_w(w):
    """Recover separable 4-tap filter f (filt = outer(f, f)) from w[0, 0];
    return a^2 where f = [a, 3a, 3a, a]."""
    filt = np.asarray(w, dtype=np.float32)[0, 0]
    j = int(np.argmax(np.abs(np.diag(filt))))
    f = filt[:, j] / np.float32(np.sqrt(filt[j, j]))
    assert np.allclose(np.outer(f, f), filt, atol=1e-5), "filter not separable"
    assert abs(f[0] - f[3]) < 1e-6 and abs(f[1] - f[2]) < 1e-6, "not symmetric"
    assert abs(f[1] - 3 * f[0]) < 1e-5, "not the 3:1 bilinear tap"
    return float(f[0]) * float(f[0])


BEST_CFG = dict(strips=[8, 16] + [24] * 4 + [8], bufs_z=4, bufs_c=3)


def _get_nc(**cfg):
    cfg = {**BEST_CFG, **cfg}
    key = tuple(sorted(
        (k, tuple(v) if isinstance(v, list) else v) for k, v in cfg.items()))
    if key not in _CACHE:
        _CACHE[key] = _build_bass(**cfg)
    return _CACHE[key]


def run_sharded(x, w, cfg=None, **run_kwargs):
    import ml_dtypes
    from concourse.bass_utils import run_bass_kernel_spmd

    scale = _taps_from_w(w)
    nc = _get_nc(**(cfg or {}))

    x = np.asarray(x, dtype=np.float32)
    in_maps = []
    for k in range(NCORES):
        xs = x[:, k * CPC:(k + 1) * CPC].reshape(NIMG, H, W)
        xk = np.zeros((NIMG, H, SW), ml_dtypes.bfloat16)
        xk[:, :, 2:2 + W] = (xs * np.float32(scale)).astype(ml_dtypes.bfloat16)
        in_maps.append({"x": xk})

    res = run_bass_kernel_spmd(nc, in_maps, core_ids=list(range(NCORES)),
                               **run_kwargs)

    full = np.empty((N, C, OH, OW), dtype=np.float32)
    for k in range(NCORES):
        e = res.results[k]["out_e"].reshape(N, CPC, OH, PW)
        o = res.results[k]["out_o"].reshape(N, CPC, OH, PW)
        # interleave column planes: out[..., 2j] = e[..., j], 2j+1 = o[..., j]
        full[:, k * CPC:(k + 1) * CPC] = np.stack(
            [e, o], axis=-1).reshape(N, CPC, OH, 2 * PW)[..., :OW].astype(
            np.float32)
    return full, res


def kernel(x, w):
    full, _ = run_sharded(x, w)
    return full
